# revision 27
# baseline (speedup 1.0000x reference)
"""Trainium2 Bass kernel for nn_LogOddsPerformanceTransformer.

Computes, for each element x of Xs:
    s   = log(x) - log(1-x)              (log-odds)
    idx = clip(floor((s - bins[0]) / step), 0, NB-1)
    out = bins[idx]

The input is staged to the device as fp16 (halves input HBM traffic; x is
capped at the largest fp16 < 1 so 1-x stays positive) and the output leaves
the device as fp16 bin values (64 distinct values in [-6, 6]; cast back to
f32 on the host).  Per Ln group (ACT is the saturated engine and only does
the two table passes):
    a  = Ln(x)            b = Ln(1-x)        (ACT, fp16 out)
per chunk (pairs of chunks are emitted pass-major so consecutive DVE
instructions belong to independent chains and the engine never stalls on
its own ack latency):
    s  = a - b                               (tensor_tensor subtract)
    t1 = s*inv + (1024 + off)     off = -b0*inv - 0.5   (integer)
    t2 = clip(t1, 1024, 1024+NB-1)   # fp16 output rounding floors to grid
    t3 = (t2 - 512) - (512 - b0*inv) # exact halves -> idx + b0*inv
    out = t3 * step
The four tensor_scalar steps hit the DVE 4x fp16 mode.  Terminal passes
(t3/out) of early chunks go to Pool; outputs stream per chunk on SP HWDGE.
Data parallel over 8 NeuronCores, 524288 elements each as [128 x 4096].
"""

import sys

sys.path.insert(0, "/opt/trn_rl_repo")

from contextlib import ExitStack

import numpy as np

import concourse.bass as bass
import concourse.tile as tile
from concourse import bacc, mybir
from concourse.bass_utils import run_bass_kernel_spmd

N = 4_194_304
NCORES = 8
NPER = N // NCORES  # 524288
P = 128
F = NPER // P  # 4096

f16 = mybir.dt.float16
f32 = mybir.dt.float32
Alu = mybir.AluOpType
Act = mybir.ActivationFunctionType

# --- tunables -------------------------------------------------------------
# chunks grouped into Ln groups; chunks in one group are emitted pass-major
LN_GROUPS = ((512,), (1024,), (1024,), (512, 512), (256, 256))
# per-pass engine schedule per chunk index (flattened): d(ve) / p(ool)
SCHED = {
    "s": "ddddddd",
    "t1": "ddddddd",
    "t2": "ddddddd",
    "t3": "ddddddd",
    "o": "ppppddd",
}
N_TRIG = 0  # this many final output DMAs go via prepared kv_writeback+trigger
TMP_BUFS = 6
# --------------------------------------------------------------------------

_BUILD_CACHE: dict[tuple, object] = {}


def _constants(bins: np.ndarray):
    """Host-side constants; None if bins don't fit the fp16 fused-floor path
    (needs uniform spacing, <= 64 bins, integer floor offset, and the unbias
    constants representable in fp16)."""
    b64 = bins.astype(np.float64)
    nb = len(bins)
    if nb > 64:
        return None
    step = np.float32((b64[-1] - b64[0]) / (nb - 1))
    inv = np.float32((nb - 1) / (b64[-1] - b64[0]))
    off = -b64[0] * float(inv) - 0.5
    uniform = np.allclose(np.diff(b64), (b64[-1] - b64[0]) / (nb - 1), rtol=0, atol=1e-5)
    C = 1024.0 + off
    HI = 1024.0 + (nb - 1)
    U2 = 512.0
    U2b = 512.0 + off + 0.5  # = 512 - b0*inv
    exact = (
        off == round(off)
        and float(np.float16(C)) == C
        and float(np.float16(U2b)) == U2b
        and abs(off) < 512
    )
    if not (uniform and exact):
        return None
    return tuple(float(v) for v in (step, inv, C, HI, U2, U2b))


def _kv_out_ap(o_d):
    """View the [128, F] dram output as kv_writeback's [1, 128, 1, F]."""
    return o_d[:].rearrange("p (a b n) -> a p b n", a=1, b=1)


def _kv_in_ap(ot):
    """View a [128, w] SBUF tile as kv_writeback's [128, 1, 1, w]."""
    return ot[:].rearrange("p (a b n) -> p a b n", a=1, b=1)


def _build(step, inv, C, HI, U2, U2b):
    chunks = [c for g in LN_GROUPS for c in g]
    assert sum(chunks) == F
    ngroups = len(LN_GROUPS)
    coff = [0]
    for c in chunks:
        coff.append(coff[-1] + c)

    nc = bacc.Bacc(
        "TRN2", target_bir_lowering=False, debug=False, num_swdge_queues=max(N_TRIG, 1)
    )
    x_d = nc.dram_tensor("x", [P, F], f16, kind="ExternalInput").ap()
    o_d = nc.dram_tensor("o", [P, F], f16, kind="ExternalOutput").ap()

    eng = {"d": nc.vector, "p": nc.gpsimd}
    trig_groups = {}  # group idx -> (queue, sem)

    with tile.TileContext(nc) as tc, ExitStack() as ctx:
        xpool = ctx.enter_context(tc.tile_pool(name="xpool", bufs=1))
        opool = ctx.enter_context(tc.tile_pool(name="opool", bufs=1))
        abpool = ctx.enter_context(tc.tile_pool(name="abpool", bufs=2))
        tmp = ctx.enter_context(tc.tile_pool(name="tmp", bufs=TMP_BUFS))
        cpool = ctx.enter_context(tc.tile_pool(name="cpool", bufs=1))

        # one input DMA per Ln group, high priority
        x_tiles = []
        with tc.high_priority():
            ci = 0
            for gi, g in enumerate(LN_GROUPS):
                lo, hi = coff[ci], coff[ci + len(g)]
                xt = xpool.tile([P, hi - lo], f16, tag=f"x{gi}", name=f"xt{gi}")
                nc.sync.dma_start(xt[:], x_d[:, lo:hi])
                x_tiles.append((xt, lo))
                ci += len(g)

        # output tiles; the last N_TRIG groups write back via SWDGE descriptors
        # prepared up front on Pool and fired by trigger_dma at completion
        # (skips HWDGE issue + DGE delay on the critical drain path)
        o_groups = []
        ci = 0
        for gi, g in enumerate(LN_GROUPS):
            lo, hi = coff[ci], coff[ci + len(g)]
            og = opool.tile([P, hi - lo], f16, tag=f"o{gi}", name=f"og{gi}")
            o_groups.append((og, lo, hi))
            ci += len(g)
        for q, gi in enumerate(range(ngroups - N_TRIG, ngroups)):
            og, lo, hi = o_groups[gi]
            idx = cpool.tile([P, 1], mybir.dt.int32, tag=f"kvidx{gi}", name=f"kvidx{gi}")
            nc.gpsimd.memset(idx[:], lo)
            sem = nc.alloc_semaphore(f"kvdma{gi}")
            data_sem = nc.alloc_semaphore(f"kvdat{gi}")
            nc.gpsimd.kv_writeback(
                _kv_out_ap(o_d),
                _kv_in_ap(og),
                idx[:],
                prepare_only=True,
                sem=sem,
                queue_num=q,
            )
            trig_groups[gi] = (q, sem, data_sem)

        ci = 0
        for gi, g in enumerate(LN_GROUPS):
            lo, hi = coff[ci], coff[ci + len(g)]
            xt, xlo = x_tiles[gi]
            xs = xt[:, lo - xlo : hi - xlo]
            a = abpool.tile([P, hi - lo], f16, tag=f"a{gi % 2}", name=f"a{gi}")
            b = abpool.tile([P, hi - lo], f16, tag=f"b{gi % 2}", name=f"b{gi}")
            nc.scalar.activation(a[:], xs, Act.Ln)
            nc.scalar.activation(b[:], xs, Act.Ln, 1.0, -1.0)

            cids = list(range(ci, ci + len(g)))
            sl = {c: (slice(None), slice(coff[c] - lo, coff[c + 1] - lo)) for c in cids}
            ts = {}
            for c in cids:  # pass-major over the group's chunks
                ts[c] = tmp.tile([P, chunks[c]], f16, tag=f"s{c % 2}", name=f"s{c}")
                eng[SCHED["s"][c]].tensor_tensor(ts[c][:], a[sl[c]], b[sl[c]], Alu.subtract)
            t1 = {}
            for c in cids:
                t1[c] = tmp.tile([P, chunks[c]], f16, tag=f"t1{c % 2}", name=f"t1{c}")
                eng[SCHED["t1"][c]].tensor_scalar(t1[c][:], ts[c][:], inv, C, Alu.mult, Alu.add)
            t2 = {}
            for c in cids:
                t2[c] = tmp.tile([P, chunks[c]], f16, tag=f"t2{c % 2}", name=f"t2{c}")
                eng[SCHED["t2"][c]].tensor_scalar(
                    t2[c][:], t1[c][:], 1024.0, HI, Alu.max, Alu.min
                )
            t3 = {}
            for c in cids:
                t3[c] = tmp.tile([P, chunks[c]], f16, tag=f"t3{c % 2}", name=f"t3{c}")
                eng[SCHED["t3"][c]].tensor_scalar(
                    t3[c][:], t2[c][:], U2, U2b, Alu.subtract, Alu.subtract
                )
            # one output tile + DMA per Ln group
            og = o_groups[gi][0]
            for c in cids:
                ins = eng[SCHED["o"][c]].tensor_scalar(
                    og[sl[c]], t3[c][:], step, None, Alu.mult
                )
                if gi in trig_groups:
                    ins.then_inc(trig_groups[gi][2], 1)
            if gi in trig_groups:
                q, _sem, data_sem = trig_groups[gi]
                nc.gpsimd.wait_ge(data_sem, len(cids))
                nc.gpsimd.trigger_dma(count=None, queue_num=q)
            else:
                nc.sync.dma_start(o_d[:, lo:hi], og[:])
            ci += len(g)

    # The tile teardown waits the SWDGE lane sems (DMASW{q}) for the deferred
    # writeback completions, but the completion increments land on the sems
    # encoded in the descriptors (kvdma{gi}): redirect those waits.  The
    # framework also adds a WAR wait on the og-tile *writers* (they follow the
    # early-emitted prep in program order, but are really the DMA's producers,
    # ordered via the explicit data_sem on the trigger): strip those.
    lane_to_sem = {f"DMASW{q}": sem for q, sem, _d in trig_groups.values()}
    kv_names = {sem.name for _q, sem, _d in trig_groups.values()}
    if lane_to_sem:
        for bb in nc.m.functions[0].blocks:
            for ins in bb.instructions:
                si = ins.sync_info
                if not si or not si.on_wait:
                    continue
                is_compute = type(ins).__name__ in (
                    "InstTensorScalarPtr",
                    "InstTensorTensor",
                    "InstActivation",
                )
                new_waits, changed = [], False
                for w in si.on_wait:
                    name = (w.ant_name or "").split("_")[0]
                    if w.sync_type == "semaphore" and name in lane_to_sem:
                        sem = lane_to_sem[name]
                        w = mybir.SyncWait(
                            sync_type="semaphore",
                            id=sem.num,
                            ant_name=sem.name,
                            wait_mode=w.wait_mode,
                            wait_value=w.wait_value,
                            wait_reg=w.wait_reg,
                        )
                        changed = True
                    if is_compute and w.sync_type == "semaphore" and w.ant_name in kv_names:
                        changed = True
                        continue  # drop the circular WAR wait
                    new_waits.append(w)
                if changed:
                    si.on_wait = new_waits

    nc.compile()
    return nc


def build(bins: np.ndarray):
    key = _constants(bins)
    if key is None:
        raise NotImplementedError("bins not supported by the fp16 fused-floor kernel")
    if key not in _BUILD_CACHE:
        _BUILD_CACHE[key] = _build(*key)
    return _BUILD_CACHE[key]


FP16_BELOW_ONE = np.float16(1.0 - 2.0**-11)


def make_in_maps(Xs: np.ndarray):
    x16 = np.minimum(Xs.astype(np.float16), FP16_BELOW_ONE)
    shards = x16.reshape(NCORES, P, F)
    return [{"x": shards[c]} for c in range(NCORES)]


def kernel(Xs: np.ndarray, bins: np.ndarray) -> np.ndarray:
    Xs = np.asarray(Xs, dtype=np.float32)
    bins = np.asarray(bins, dtype=np.float32)
    nc = build(bins)
    res = run_bass_kernel_spmd(nc, make_in_maps(Xs), core_ids=list(range(NCORES)))
    out = np.concatenate([r["o"].reshape(-1) for r in res.results])
    return out.astype(np.float32)


# revision 31
# speedup vs baseline: 1.0050x; 1.0050x over previous
"""Trainium2 Bass kernel for nn_LogOddsPerformanceTransformer.

Computes, for each element x of Xs:
    s   = log(x) - log(1-x)              (log-odds)
    idx = clip(floor((s - bins[0]) / step), 0, NB-1)
    out = bins[idx]

The input is staged to the device as fp16 (halves input HBM traffic; x is
capped at the largest fp16 < 1 so 1-x stays positive) and the output leaves
the device as fp16 bin values (64 distinct values in [-6, 6]; cast back to
f32 on the host).  Per Ln group (ACT is the saturated engine and only does
the two table passes):
    a  = Ln(x)            b = Ln(1-x)        (ACT, fp16 out)
per chunk (pairs of chunks are emitted pass-major so consecutive DVE
instructions belong to independent chains and the engine never stalls on
its own ack latency):
    s  = a - b                               (tensor_tensor subtract)
    t1 = s*inv + (1024 + off)     off = -b0*inv - 0.5   (integer)
    t2 = clip(t1, 1024, 1024+NB-1)   # fp16 output rounding floors to grid
    t3 = (t2 - 512) - (512 - b0*inv) # exact halves -> idx + b0*inv
    out = t3 * step
The four tensor_scalar steps hit the DVE 4x fp16 mode.  Terminal passes
(t3/out) of early chunks go to Pool; outputs stream per chunk on SP HWDGE.
Data parallel over 8 NeuronCores, 524288 elements each as [128 x 4096].
"""

import sys

sys.path.insert(0, "/opt/trn_rl_repo")

from contextlib import ExitStack

import numpy as np

import concourse.bass as bass
import concourse.tile as tile
from concourse import bacc, mybir
from concourse.bass_utils import run_bass_kernel_spmd

N = 4_194_304
NCORES = 8
NPER = N // NCORES  # 524288
P = 128
F = NPER // P  # 4096

f16 = mybir.dt.float16
f32 = mybir.dt.float32
Alu = mybir.AluOpType
Act = mybir.ActivationFunctionType

# --- tunables -------------------------------------------------------------
# chunks grouped into Ln groups; chunks in one group are emitted pass-major
LN_GROUPS = ((512,), (1024,), (1024,), (512, 512), (256, 256))
# per-pass engine schedule per chunk index (flattened): d(ve) / p(ool)
SCHED = {
    "s": "ddddddd",
    "t1": "ddddddd",
    "t2": "ddddddd",
    "t3": "ddddadd",
    "o": "ppppddd",
}
N_TRIG = 0  # this many final output DMAs go via prepared kv_writeback+trigger
TMP_BUFS = 6
# --------------------------------------------------------------------------

_BUILD_CACHE: dict[tuple, object] = {}


def _constants(bins: np.ndarray):
    """Host-side constants; None if bins don't fit the fp16 fused-floor path
    (needs uniform spacing, <= 64 bins, integer floor offset, and the unbias
    constants representable in fp16)."""
    b64 = bins.astype(np.float64)
    nb = len(bins)
    if nb > 64:
        return None
    step = np.float32((b64[-1] - b64[0]) / (nb - 1))
    inv = np.float32((nb - 1) / (b64[-1] - b64[0]))
    off = -b64[0] * float(inv) - 0.5
    uniform = np.allclose(np.diff(b64), (b64[-1] - b64[0]) / (nb - 1), rtol=0, atol=1e-5)
    C = 1024.0 + off
    HI = 1024.0 + (nb - 1)
    U2 = 512.0
    U2b = 512.0 + off + 0.5  # = 512 - b0*inv
    exact = (
        off == round(off)
        and float(np.float16(C)) == C
        and float(np.float16(U2b)) == U2b
        and abs(off) < 512
    )
    if not (uniform and exact):
        return None
    return tuple(float(v) for v in (step, inv, C, HI, U2, U2b))


def _kv_out_ap(o_d):
    """View the [128, F] dram output as kv_writeback's [1, 128, 1, F]."""
    return o_d[:].rearrange("p (a b n) -> a p b n", a=1, b=1)


def _kv_in_ap(ot):
    """View a [128, w] SBUF tile as kv_writeback's [128, 1, 1, w]."""
    return ot[:].rearrange("p (a b n) -> p a b n", a=1, b=1)


def _build(step, inv, C, HI, U2, U2b):
    chunks = [c for g in LN_GROUPS for c in g]
    assert sum(chunks) == F
    ngroups = len(LN_GROUPS)
    coff = [0]
    for c in chunks:
        coff.append(coff[-1] + c)

    nc = bacc.Bacc(
        "TRN2", target_bir_lowering=False, debug=False, num_swdge_queues=max(N_TRIG, 1)
    )
    x_d = nc.dram_tensor("x", [P, F], f16, kind="ExternalInput").ap()
    o_d = nc.dram_tensor("o", [P, F], f16, kind="ExternalOutput").ap()

    eng = {"d": nc.vector, "p": nc.gpsimd}
    trig_groups = {}  # group idx -> (queue, sem)

    with tile.TileContext(nc) as tc, ExitStack() as ctx:
        xpool = ctx.enter_context(tc.tile_pool(name="xpool", bufs=1))
        opool = ctx.enter_context(tc.tile_pool(name="opool", bufs=1))
        abpool = ctx.enter_context(tc.tile_pool(name="abpool", bufs=2))
        tmp = ctx.enter_context(tc.tile_pool(name="tmp", bufs=TMP_BUFS))
        cpool = ctx.enter_context(tc.tile_pool(name="cpool", bufs=1))

        # one input DMA per Ln group, high priority
        x_tiles = []
        with tc.high_priority():
            ci = 0
            for gi, g in enumerate(LN_GROUPS):
                lo, hi = coff[ci], coff[ci + len(g)]
                xt = xpool.tile([P, hi - lo], f16, tag=f"x{gi}", name=f"xt{gi}")
                nc.sync.dma_start(xt[:], x_d[:, lo:hi])
                x_tiles.append((xt, lo))
                ci += len(g)

        # output tiles; the last N_TRIG groups write back via SWDGE descriptors
        # prepared up front on Pool and fired by trigger_dma at completion
        # (skips HWDGE issue + DGE delay on the critical drain path)
        o_groups = []
        ci = 0
        for gi, g in enumerate(LN_GROUPS):
            lo, hi = coff[ci], coff[ci + len(g)]
            og = opool.tile([P, hi - lo], f16, tag=f"o{gi}", name=f"og{gi}")
            o_groups.append((og, lo, hi))
            ci += len(g)
        for q, gi in enumerate(range(ngroups - N_TRIG, ngroups)):
            og, lo, hi = o_groups[gi]
            idx = cpool.tile([P, 1], mybir.dt.int32, tag=f"kvidx{gi}", name=f"kvidx{gi}")
            nc.gpsimd.memset(idx[:], lo)
            sem = nc.alloc_semaphore(f"kvdma{gi}")
            data_sem = nc.alloc_semaphore(f"kvdat{gi}")
            nc.gpsimd.kv_writeback(
                _kv_out_ap(o_d),
                _kv_in_ap(og),
                idx[:],
                prepare_only=True,
                sem=sem,
                queue_num=q,
            )
            trig_groups[gi] = (q, sem, data_sem)

        ci = 0
        for gi, g in enumerate(LN_GROUPS):
            lo, hi = coff[ci], coff[ci + len(g)]
            xt, xlo = x_tiles[gi]
            xs = xt[:, lo - xlo : hi - xlo]
            a = abpool.tile([P, hi - lo], f16, tag=f"a{gi % 2}", name=f"a{gi}")
            b = abpool.tile([P, hi - lo], f16, tag=f"b{gi % 2}", name=f"b{gi}")
            nc.scalar.activation(a[:], xs, Act.Ln)
            nc.scalar.activation(b[:], xs, Act.Ln, 1.0, -1.0)

            cids = list(range(ci, ci + len(g)))
            sl = {c: (slice(None), slice(coff[c] - lo, coff[c + 1] - lo)) for c in cids}
            ts = {}
            for c in cids:  # pass-major over the group's chunks
                ts[c] = tmp.tile([P, chunks[c]], f16, tag=f"s{c % 2}", name=f"s{c}")
                eng[SCHED["s"][c]].tensor_tensor(ts[c][:], a[sl[c]], b[sl[c]], Alu.subtract)
            t1 = {}
            for c in cids:
                t1[c] = tmp.tile([P, chunks[c]], f16, tag=f"t1{c % 2}", name=f"t1{c}")
                e = SCHED["t1"][c]
                if e == "a":
                    nc.scalar.activation(t1[c][:], ts[c][:], Act.Copy, C, inv)
                else:
                    eng[e].tensor_scalar(t1[c][:], ts[c][:], inv, C, Alu.mult, Alu.add)
            t2 = {}
            for c in cids:
                t2[c] = tmp.tile([P, chunks[c]], f16, tag=f"t2{c % 2}", name=f"t2{c}")
                eng[SCHED["t2"][c]].tensor_scalar(
                    t2[c][:], t1[c][:], 1024.0, HI, Alu.max, Alu.min
                )
            t3 = {}
            for c in cids:
                t3[c] = tmp.tile([P, chunks[c]], f16, tag=f"t3{c % 2}", name=f"t3{c}")
                e = SCHED["t3"][c]
                if e == "a":
                    # ACT's affine is f32-internal, so one fused subtract is exact
                    nc.scalar.activation(t3[c][:], t2[c][:], Act.Copy, -(U2 + U2b), 1.0)
                else:
                    eng[e].tensor_scalar(
                        t3[c][:], t2[c][:], U2, U2b, Alu.subtract, Alu.subtract
                    )
            # one output tile + DMA per Ln group
            og = o_groups[gi][0]
            for c in cids:
                e = SCHED["o"][c]
                if e == "a":
                    ins = nc.scalar.activation(og[sl[c]], t3[c][:], Act.Copy, 0.0, step)
                else:
                    ins = eng[e].tensor_scalar(og[sl[c]], t3[c][:], step, None, Alu.mult)
                if gi in trig_groups:
                    ins.then_inc(trig_groups[gi][2], 1)
            if gi in trig_groups:
                q, _sem, data_sem = trig_groups[gi]
                nc.gpsimd.wait_ge(data_sem, len(cids))
                nc.gpsimd.trigger_dma(count=None, queue_num=q)
            else:
                nc.sync.dma_start(o_d[:, lo:hi], og[:])
            ci += len(g)

    # The tile teardown waits the SWDGE lane sems (DMASW{q}) for the deferred
    # writeback completions, but the completion increments land on the sems
    # encoded in the descriptors (kvdma{gi}): redirect those waits.  The
    # framework also adds a WAR wait on the og-tile *writers* (they follow the
    # early-emitted prep in program order, but are really the DMA's producers,
    # ordered via the explicit data_sem on the trigger): strip those.
    lane_to_sem = {f"DMASW{q}": sem for q, sem, _d in trig_groups.values()}
    kv_names = {sem.name for _q, sem, _d in trig_groups.values()}
    if lane_to_sem:
        for bb in nc.m.functions[0].blocks:
            for ins in bb.instructions:
                si = ins.sync_info
                if not si or not si.on_wait:
                    continue
                is_compute = type(ins).__name__ in (
                    "InstTensorScalarPtr",
                    "InstTensorTensor",
                    "InstActivation",
                )
                new_waits, changed = [], False
                for w in si.on_wait:
                    name = (w.ant_name or "").split("_")[0]
                    if w.sync_type == "semaphore" and name in lane_to_sem:
                        sem = lane_to_sem[name]
                        w = mybir.SyncWait(
                            sync_type="semaphore",
                            id=sem.num,
                            ant_name=sem.name,
                            wait_mode=w.wait_mode,
                            wait_value=w.wait_value,
                            wait_reg=w.wait_reg,
                        )
                        changed = True
                    if is_compute and w.sync_type == "semaphore" and w.ant_name in kv_names:
                        changed = True
                        continue  # drop the circular WAR wait
                    new_waits.append(w)
                if changed:
                    si.on_wait = new_waits

    nc.compile()
    return nc


def build(bins: np.ndarray):
    key = _constants(bins)
    if key is None:
        raise NotImplementedError("bins not supported by the fp16 fused-floor kernel")
    if key not in _BUILD_CACHE:
        _BUILD_CACHE[key] = _build(*key)
    return _BUILD_CACHE[key]


FP16_BELOW_ONE = np.float16(1.0 - 2.0**-11)


def make_in_maps(Xs: np.ndarray):
    x16 = np.minimum(Xs.astype(np.float16), FP16_BELOW_ONE)
    shards = x16.reshape(NCORES, P, F)
    return [{"x": shards[c]} for c in range(NCORES)]


def kernel(Xs: np.ndarray, bins: np.ndarray) -> np.ndarray:
    Xs = np.asarray(Xs, dtype=np.float32)
    bins = np.asarray(bins, dtype=np.float32)
    nc = build(bins)
    res = run_bass_kernel_spmd(nc, make_in_maps(Xs), core_ids=list(range(NCORES)))
    out = np.concatenate([r["o"].reshape(-1) for r in res.results])
    return out.astype(np.float32)
